# revision 33
# baseline (speedup 1.0000x reference)
"""HardAttention kernel for Trainium2 (8 NeuronCores, Bass/Tile).

reference:
    scores = einsum("btd,bcsd->btcs", xs, ys)   # (B,Tx,C,Ty)
    out    = scores.max(-1).sum(1)              # (B,C)

Shapes: B=16, Tx=128, C=64, Ty=128, d=768.

Strategy:
  - Data-parallel over B: core i handles batches [2i, 2i+2).
  - Host pre-arranges both operands d-major (PE contracts over the
    partition axis) and pre-casts to fp8e4 (e4m3), so every device DMA is
    a same-dtype HWDGE transfer with fully contiguous partition lines:
        xsP[p, b, k, t]       = xs[b, t, 128k+p]        (128, B, KC, TX)
        ysP[b, q, p, k, c, s] = ys[b, 16q+c, s, 128k+p] (B, NQ, 128, KC, QC, TY)
  - Per (b, quarter-of-16-candidates): three k-pair chunk DMAs with fully
    contiguous 4KB partition lines (split finely so the first matmul
    starts ~1us after the first 0.5MB lands, and spread over both HWDGE
    queues so all 16 DMA engines stream at ~420 GB/s), then fp8 DoubleRow
    matmuls (K_eff=256 per pass, 3 passes over d, 2 rows/cycle) into 4
    PSUM banks; DVE reduce_max over Ty per candidate into an SBUF tile
    m[t, b, c]; per-b ones-vector matmul contracts the partition axis
    (sum over t) -> out[b, c].

Measured: ~49.7-50.5us HW exec (core 0, unthrottled), vs 9.6ms baseline.
Roofline: 12.6MB fp8 ys-slice per core at ~430 GB/s = 29us stream +
~7us framework preamble + ~3us DMA spin-up + ~5us tail/epilogue.
"""

import os
import numpy as np
import ml_dtypes

B, TX, C, TY, D = 16, 128, 64, 128, 768
N_CORES = 8
BPC = B // N_CORES          # batches per core = 2
KC = D // 128               # contraction chunks = 6
QC = 16                     # candidates per slab
NQ = C // QC                # slabs per batch = 4
G = 4                       # candidates per matmul (N = G*TY = 512)

# "fp8": e4m3 operands, DoubleRow matmuls (2 d-chunks per pass).
# "bf16": bfloat16 operands, plain matmuls. Fallback if fp8 misbehaves.
MODE = os.environ.get("HA_MODE", "fp8")
SPLIT_K = os.environ.get("HA_SPLIT_K", "1") == "1"  # k-pair DMAs vs whole slab
PER_B = os.environ.get("HA_PERB", "1") == "1"  # final matmul per batch vs combined
YBUFS = int(os.environ.get("HA_YBUFS", "16"))
PS2 = os.environ.get("HA_PS2", "0") == "1"  # 2-bank psum tiles, fused reduces
TAIL_SCHED = os.environ.get("HA_TAIL_SCHED", "0") == "1"  # prefetch last slabs' early k-pairs

_CACHE = {}
LAST_RESULT = None  # BassKernelResults of the most recent device run


def _np_dt():
    return ml_dtypes.float8_e4m3 if MODE == "fp8" else ml_dtypes.bfloat16


def _build():
    import concourse.bass as bass
    import concourse.mybir as mybir
    import concourse.tile as tile
    from concourse import bacc

    mm_dt = mybir.dt.float8e4 if MODE == "fp8" else mybir.dt.bfloat16
    f32 = mybir.dt.float32

    nc = bacc.Bacc(
        "TRN2",
        target_bir_lowering=False,
        debug=False,
        num_devices=N_CORES,
    )

    xs_ap = nc.dram_tensor("xsP", (128, BPC, KC, TX), mm_dt, kind="ExternalInput").ap()
    ys_ap = nc.dram_tensor(
        "ysP", (BPC, NQ, 128, KC, QC, TY), mm_dt, kind="ExternalInput"
    ).ap()
    out_ap = nc.dram_tensor("out", (1, BPC * C), f32, kind="ExternalOutput").ap()

    with tile.TileContext(nc) as tc:
        with (
            tc.tile_pool(name="xt", bufs=1) as xpool,
            tc.tile_pool(name="yt", bufs=YBUFS) as ypool,
            tc.tile_pool(name="mt", bufs=1) as mpool,
            tc.tile_pool(name="ones", bufs=1) as opool,
            tc.tile_pool(name="osb", bufs=2) as obpool,
            tc.tile_pool(name="ps", bufs=4 if (PS2 and MODE == "fp8") else 7, space="PSUM") as pspool,
            tc.tile_pool(name="pso", bufs=1, space="PSUM") as psopool,
        ):
            # All of xsP for this core: [p, b, k, t], fully contiguous.
            xt = xpool.tile([128, BPC, KC, TX], mm_dt)
            nc.scalar.dma_start(xt[:], xs_ap[:])

            ones = opool.tile([128, 1], f32)
            nc.any.memset(ones[:], 1.0)

            # max_s scores, [t, b, c]
            m = mpool.tile([128, BPC, C], f32)

            qeng = [nc.sync, nc.scalar]
            nq_dma = 0
            KP = KC // 2  # k-pair chunks per slab = 3

            ytiles = {}
            if MODE == "fp8" and SPLIT_K:
                # k-pair chunk DMAs: [p, 2, c, s] — contiguous partition
                # lines, fine-grained so matmuls start after the first
                # 0.5MB instead of the full slab. Issue order pulls the
                # final two slabs' early k-pairs to the stream FRONT so
                # only the last slab's stop-matmuls + reduces trail the
                # last byte (instead of a 2-slab matmul/reduce burst).
                if TAIL_SCHED:
                    early = [
                        (BPC - 1, NQ - 1, 0),
                        (BPC - 1, NQ - 1, 1),
                        (BPC - 1, NQ - 2, 0),
                        (BPC - 1, NQ - 2, 1),
                        (BPC - 1, NQ - 2, 2),
                    ]
                    sched = early + [
                        (b, q, kk)
                        for b in range(BPC)
                        for q in range(NQ)
                        for kk in range(KP)
                        if (b, q, kk) not in early
                        and (b, q, kk) != (BPC - 1, NQ - 1, 2)
                    ] + [(BPC - 1, NQ - 1, 2)]
                else:
                    sched = [
                        (b, q, kk)
                        for b in range(BPC)
                        for q in range(NQ)
                        for kk in range(KP)
                    ]
                for b, q, kk in sched:
                    yt = ypool.tile([128, 2, QC, TY], mm_dt)
                    qeng[nq_dma % 2].dma_start(
                        yt[:], ys_ap[b, q, :, 2 * kk : 2 * kk + 2]
                    )
                    nq_dma += 1
                    ytiles[(b, q, kk)] = yt

            for b in range(BPC):
                for q in range(NQ):
                    if MODE == "fp8" and SPLIT_K:
                        rhs_of = lambda kk, g, b=b, q=q: ytiles[(b, q, kk)][
                            :, :, g * G : (g + 1) * G, :
                        ]
                    else:
                        yt = ypool.tile([128, KC, QC, TY], mm_dt)
                        qeng[nq_dma % 2].dma_start(yt[:], ys_ap[b, q])
                        nq_dma += 1
                        rhs_of = lambda kk, g: yt[
                            :, 2 * kk : 2 * kk + 2, g * G : (g + 1) * G, :
                        ]
                    if PS2 and MODE == "fp8":
                        # Two 2-bank psum tiles per slab; one fused reduce per
                        # tile halves DVE instruction+sem overhead, giving the
                        # DVE slack on clock-throttled cores.
                        pstiles = [
                            pspool.tile(
                                [128, 2, G, TY], f32, name=f"ps_{b}_{q}_{h}", tag="ps"
                            )
                            for h in range(2)
                        ]
                        out_of = lambda g: pstiles[g // 2][:, g % 2]
                        last_pstile = pstiles[1]
                    else:
                        psums = [
                            pspool.tile(
                                [128, G, TY], f32, name=f"ps_{b}_{q}_{g}", tag="ps"
                            )
                            for g in range(G)
                        ]
                        out_of = lambda g: psums[g][:]
                    if MODE == "fp8":
                        # kk-outer: after the last k-pair lands, only the
                        # G stop-matmuls remain on the in-order PE queue.
                        for kk in range(KP):
                            for g in range(G):
                                nc.tensor.matmul(
                                    out_of(g),
                                    lhsT=xt[:, b, 2 * kk : 2 * kk + 2, :],
                                    rhs=rhs_of(kk, g),
                                    start=(kk == 0),
                                    stop=(kk == KP - 1),
                                    perf_mode=mybir.MatmulPerfMode.DoubleRow,
                                )
                    else:
                        for k in range(KC):
                            for g in range(G):
                                nc.tensor.matmul(
                                    out_of(g),
                                    lhsT=xt[:, b, k, :],
                                    rhs=yt[:, k, g * G : (g + 1) * G, :],
                                    start=(k == 0),
                                    stop=(k == KC - 1),
                                )
                    if PS2 and MODE == "fp8":
                        for h in range(2):
                            cc = q * QC + h * (QC // 2)
                            nc.vector.reduce_max(
                                m[:, b, cc : cc + QC // 2],
                                pstiles[h][:],
                                axis=mybir.AxisListType.X,
                            )
                    else:
                        for g in range(G):
                            cc = q * QC + g * G
                            nc.vector.reduce_max(
                                m[:, b, cc : cc + G],
                                psums[g][:],
                                axis=mybir.AxisListType.X,
                            )
                if PER_B:
                    # sum over t (partition axis) via ones-vector matmul;
                    # per-b so only the last batch's (tiny) chain is on the
                    # critical tail. With PS2 all 8 banks belong to the slab
                    # pool, so the accumulator is carved from a corner of
                    # this b's last slab tile (after its reduce).
                    if PS2 and MODE == "fp8":
                        out_ps = last_pstile[0:1, 0, 0, 0:C]
                    else:
                        out_ps_t = psopool.tile(
                            [1, C], f32, name="out_ps", tag="out_ps"
                        )
                        out_ps = out_ps_t[:]
                    nc.tensor.matmul(
                        out_ps, lhsT=ones[:], rhs=m[:, b, :], start=True, stop=True
                    )
                    osb = obpool.tile([1, C], f32, tag="osb")
                    nc.vector.tensor_copy(osb[:], out_ps)
                    nc.sync.dma_start(out_ap[0, b * C : (b + 1) * C], osb[:])
            if not PER_B:
                out_ps = psopool.tile([1, BPC * C], f32, tag="out_ps")
                nc.tensor.matmul(
                    out_ps[:], lhsT=ones[:], rhs=m[:], start=True, stop=True
                )
                osb = obpool.tile([1, BPC * C], f32, tag="osb")
                nc.vector.tensor_copy(osb[:], out_ps[:])
                nc.sync.dma_start(out_ap[:], osb[:])

    nc.compile()
    return nc


def _get_nc():
    key = ("nc", MODE, SPLIT_K, PER_B, YBUFS, PS2, TAIL_SCHED)
    if key not in _CACHE:
        _CACHE[key] = _build()
    return _CACHE[key]


def _prep(xs: np.ndarray, ys: np.ndarray):
    """Host-side layout: d-major, blocked by 128-chunks of d, cast to mm dtype."""
    np_dt = _np_dt()
    xs = np.asarray(xs, dtype=np.float32)
    ys = np.asarray(ys, dtype=np.float32)
    # xsP[p, b, k, t] = xs[b, t, 128k+p]
    xsP = np.ascontiguousarray(
        xs.astype(np_dt).reshape(B, TX, KC, 128).transpose(3, 0, 2, 1)
    )
    # ysP[b, q, p, k, c, s] = ys[b, 16q+c, s, 128k+p]
    ysP = np.ascontiguousarray(
        ys.astype(np_dt).reshape(B, NQ, QC, TY, KC, 128).transpose(0, 1, 5, 4, 2, 3)
    )
    return xsP, ysP


def kernel(xs: np.ndarray, ys: np.ndarray) -> np.ndarray:
    global LAST_RESULT
    from concourse.bass_utils import run_bass_kernel_spmd

    nc = _get_nc()
    xsP, ysP = _prep(xs, ys)
    in_maps = [
        {
            "xsP": np.ascontiguousarray(xsP[:, i * BPC : (i + 1) * BPC]),
            "ysP": ysP[i * BPC : (i + 1) * BPC],
        }
        for i in range(N_CORES)
    ]
    res = run_bass_kernel_spmd(nc, in_maps, core_ids=list(range(N_CORES)))
    LAST_RESULT = res
    out = np.concatenate(
        [res.results[i]["out"].reshape(BPC, C) for i in range(N_CORES)], axis=0
    )
    return out.astype(np.float32)
